# revision 6
# baseline (speedup 1.0000x reference)
"""Sliding-window GQA self-attention (B=2,T=2048,E=2048,H=16,KV=4,D=128,W=512)
on 8 Trainium2 NeuronCores.

Sharding: sequence-parallel. Core c owns 512 query rows (batch c//4, quarter
c%4) and receives a 512-row key/value halo (zero-padded before the sequence
start; padded keys contribute exactly exp(0)=1 to the softmax denominator,
which is subtracted out via a precomputed correction).

v2 dataflow (per core):
  - Q/K projections run in fp8e4m3 with DoubleRow perf mode (K=256 per pass,
    2x PE throughput). Weights are pre-scaled by 512 host-side so they sit in
    the e4m3 normal range; the 1/512 is folded into the RoPE tables. Scores
    are tiny (|s|~0.02) and softmax is shift/scale tolerant, so fp8 q/k error
    is negligible. V/attn/out-proj stay bf16.
  - RoPE via 4 DVE ops: two full-width [128,n] muls against duplicated
    cos/sign-folded-sin tables (bf16 out), then two half-width bf16 subs
    (2x DVE rate).
  - Softmax denominators per query block: 20 M=1 ones-matmuls col-tiled via
    tile_position=(0,32g), 4 kv-groups running concurrently in separate PE
    column groups; pad-correction folded in with one fused DVE
    scalar_tensor_tensor; one 128-wide fast reciprocal.
  - Engine balance: exp + att-PSUM-drain on ScalarE, rope/masks/den/out-drain
    on VectorE, normalize muls + broadcast on GpSimd.
  - DMA on three parallel queues (sync HWDGE, scalar HWDGE, gpsimd SWDGE)
    with fully contiguous host-side layouts; output in bf16 (host casts back
    to f32).
"""

import numpy as np
import ml_dtypes

import concourse.bass as bass
import concourse.bacc as bacc
import concourse.mybir as mybir
import concourse.tile as tile
from concourse.bass_utils import run_bass_kernel_spmd

BF16 = ml_dtypes.bfloat16
FP8 = ml_dtypes.float8_e4m3

B, T, E = 2, 2048, 2048
H, KV, D = 16, 4, 128
NREP = H // KV  # 4 query heads per kv head
WINDOW = 512
THETA = 10000.0
WS = 512.0  # fp8 weight pre-scale (power of two; folded into rope tables)

NCORES = 8
Q = 512          # owned query rows per core
TH = Q + WINDOW  # rows incl. halo = 1024
EC = E // 128    # 16 e-chunks
PP = EC // 2     # 8 DoubleRow passes (K=256 each)
NQB = Q // 128   # 4 query blocks per core
NJ = 5           # key blocks per query block (window 512 + diag)
F32 = mybir.dt.float32
BF = mybir.dt.bfloat16
F8 = mybir.dt.float8e4

_CACHE = {}


def _build_bass():
    nc = bacc.Bacc("TRN2", target_bir_lowering=False, debug=False,
                   enable_asserts=True, num_devices=NCORES)

    xt8_d = nc.dram_tensor("xt8", [128, EC, TH], F8, kind="ExternalInput")
    xtb_d = nc.dram_tensor("xtb", [4, 128, EC, 256], BF, kind="ExternalInput")
    wk_d = nc.dram_tensor("wk8", [128, EC, KV, 128], F8, kind="ExternalInput")
    wq_d = nc.dram_tensor("wq8", [H, 128, EC, 128], F8, kind="ExternalInput")
    wv_d = nc.dram_tensor("wv", [128, EC, KV * 128], BF, kind="ExternalInput")
    wo_d = nc.dram_tensor("wo", [4, 128, H, 512], BF, kind="ExternalInput")
    ck_d = nc.dram_tensor("ck", [128, TH], F32, kind="ExternalInput")
    sk_d = nc.dram_tensor("sk", [128, TH], F32, kind="ExternalInput")
    cq_d = nc.dram_tensor("cq", [128, Q], F32, kind="ExternalInput")
    sq_d = nc.dram_tensor("sq", [128, Q], F32, kind="ExternalInput")
    m0_d = nc.dram_tensor("mask0", [128, 512], BF, kind="ExternalInput")
    m4_d = nc.dram_tensor("mask4", [128, 512], BF, kind="ExternalInput")
    corr_d = nc.dram_tensor("corr", [128, NQB * 512], F32, kind="ExternalInput")
    out_d = nc.dram_tensor("out", [Q, E], BF, kind="ExternalOutput")

    EXP = mybir.ActivationFunctionType.Exp
    COPY = mybir.ActivationFunctionType.Copy
    DR = mybir.MatmulPerfMode.DoubleRow
    MULT = mybir.AluOpType.mult
    SUB = mybir.AluOpType.subtract

    with tile.TileContext(nc) as tc:
        with (
            tc.tile_pool(name="const", bufs=1) as const,
            tc.tile_pool(name="ps_proj", bufs=2, space="PSUM") as ps_proj,
            tc.tile_pool(name="ps_sc", bufs=3, space="PSUM") as ps_scp,
            tc.tile_pool(name="ps_att", bufs=2, space="PSUM") as ps_attp,
            tc.tile_pool(name="ps_den", bufs=1, space="PSUM") as ps_denp,
        ):
            # ---- persistent tensors ----
            ones_b = const.tile([128, 1], BF, name="ones_b")
            nc.vector.memset(ones_b, 1.0)
            zero_b = const.tile([128, 1], F32, name="zero_b")
            nc.vector.memset(zero_b, 0.0)
            kT = [const.tile([128, TH], BF, tag=f"kT{g}", name=f"kT{g}")
                  for g in range(KV)]
            v_sb = [const.tile([128, KV * 128], BF, tag=f"v{tv}", name=f"v{tv}")
                    for tv in range(TH // 128)]
            qT = [const.tile([128, NREP, Q], BF, tag=f"qT{g}", name=f"qT{g}")
                  for g in range(KV)]

            # ---- projection phase ----
            with (
                tc.tile_pool(name="xtp", bufs=1) as xtp,
                tc.tile_pool(name="wqp", bufs=2) as wqp,
                tc.tile_pool(name="ropet", bufs=3) as ropet,
            ):
                # sync HWDGE queue: k-projection critical path, then xtb
                wk_t = xtp.tile([128, EC, KV, 128], F8, name="wk_t")
                nc.sync.dma_start(out=wk_t, in_=wk_d[:, :, :, :])
                xt8 = xtp.tile([128, EC, TH], F8, name="xt8")
                for eg in range(4):
                    nc.sync.dma_start(out=xt8[:, eg * 4:(eg + 1) * 4, :],
                                      in_=xt8_d[:, eg * 4:(eg + 1) * 4, :])
                ck = xtp.tile([128, TH], F32, name="ck")
                nc.sync.dma_start(out=ck, in_=ck_d[:, :])
                sk = xtp.tile([128, TH], F32, name="sk")
                nc.sync.dma_start(out=sk, in_=sk_d[:, :])
                cq = xtp.tile([128, Q], F32, name="cq")
                nc.sync.dma_start(out=cq, in_=cq_d[:, :])
                sq = xtp.tile([128, Q], F32, name="sq")
                nc.sync.dma_start(out=sq, in_=sq_d[:, :])
                xtb = xtp.tile([128, EC, TH], BF, name="xtb")
                for tg in range(4):
                    nc.sync.dma_start(out=xtb[:, :, tg * 256:(tg + 1) * 256],
                                      in_=xtb_d[tg, :, :, :])

                # scalar HWDGE queue (parallel): q/v weights
                wq_t = [wqp.tile([128, EC, 128], F8, tag="wq", name=f"wq{h}")
                        for h in range(2)]
                nc.scalar.dma_start(out=wq_t[0], in_=wq_d[0, :, :, :])
                nc.scalar.dma_start(out=wq_t[1], in_=wq_d[1, :, :, :])
                wv_t = xtp.tile([128, EC, KV * 128], BF, name="wv_t")
                nc.scalar.dma_start(out=wv_t, in_=wv_d[:, :, :])

                def rope(dst, ps, cos_ap, sin_ap, n):
                    """dst[128, n] bf16 <- rope(ps[128, n] f32 PSUM).

                    cos_ap is cos duplicated into both halves; sin_ap is
                    [+sin; -sin]. tb is built partition-swapped (legal: the
                    swapped operand is the PSUM input; the one SBUF input of
                    each mul is partition-aligned with its output), so the
                    final combine is a single full-width aligned subtract."""
                    ta = ropet.tile([128, n], BF, tag="ta", name="ta")
                    nc.vector.tensor_mul(ta, ps, cos_ap)
                    tb = ropet.tile([128, n], BF, tag="tb", name="tb")
                    nc.vector.tensor_mul(tb[0:64, :], ps[64:128, :], sin_ap[0:64, :])
                    nc.vector.tensor_mul(tb[64:128, :], ps[0:64, :], sin_ap[64:128, :])
                    nc.vector.tensor_sub(dst, ta, tb)

                # k projection (fp8 DoubleRow) + rope
                for th in range(TH // 512):
                    sl = slice(th * 512, (th + 1) * 512)
                    for g in range(KV):
                        ps = ps_proj.tile([128, 512], F32, tag="proj", name="psk")
                        for pp in range(PP):
                            nc.tensor.matmul(
                                ps, wk_t[:, 2 * pp:2 * pp + 2, g, :],
                                xt8[:, 2 * pp:2 * pp + 2, sl],
                                start=(pp == 0), stop=(pp == PP - 1),
                                perf_mode=DR)
                        rope(kT[g][:, sl], ps, ck[:, sl], sk[:, sl], 512)

                # q projection (fp8 DoubleRow) + rope; scale folded into cq/sq
                for g in range(KV):
                    for hg in range(NREP):
                        h = g * NREP + hg
                        if h >= 2:
                            wq_t[h % 2] = wqp.tile([128, EC, 128], F8, tag="wq",
                                                   name=f"wq{h}")
                            nc.scalar.dma_start(out=wq_t[h % 2],
                                                in_=wq_d[h, :, :, :])
                        ps = ps_proj.tile([128, 512], F32, tag="proj", name="psq")
                        for pp in range(PP):
                            nc.tensor.matmul(
                                ps, wq_t[h % 2][:, 2 * pp:2 * pp + 2, :],
                                xt8[:, 2 * pp:2 * pp + 2, WINDOW:TH],
                                start=(pp == 0), stop=(pp == PP - 1),
                                perf_mode=DR)
                        rope(qT[g][:, hg, :], ps, cq, sq, 512)

                # v projection (bf16)
                for tv in range(TH // 128):
                    sl = slice(tv * 128, (tv + 1) * 128)
                    ps = ps_proj.tile([128, 512], F32, tag="proj", name="psv")
                    for ec in range(EC):
                        nc.tensor.matmul(ps, xtb[:, ec, sl], wv_t[:, ec, :],
                                         start=(ec == 0), stop=(ec == EC - 1))
                    nc.scalar.activation(v_sb[tv], ps, COPY)

            # ---- attention + output projection ----
            with (
                tc.tile_pool(name="probs", bufs=25) as probsp,
                tc.tile_pool(name="attu", bufs=6) as attup,
                tc.tile_pool(name="bcp", bufs=3) as bcp,
                tc.tile_pool(name="small", bufs=2) as small,
                tc.tile_pool(name="attsb", bufs=1) as attsbp,
                tc.tile_pool(name="wop", bufs=1) as wop,
                tc.tile_pool(name="obufp", bufs=3) as obufp,
            ):
                # gpsimd SWDGE queue: masks + corr first (needed at first
                # attention block), then wo (needed at out-proj)
                m0 = const.tile([128, 512], BF, name="m0")
                nc.gpsimd.dma_start(out=m0, in_=m0_d[:, :])
                m4 = const.tile([128, 512], BF, name="m4")
                nc.gpsimd.dma_start(out=m4, in_=m4_d[:, :])
                corr = const.tile([128, NQB * 512], F32, name="corr")
                nc.gpsimd.dma_start(out=corr, in_=corr_d[:, :])
                wo_t = [wop.tile([128, H, 512], BF, tag=f"wo{ec}",
                                 name=f"wo{ec}") for ec in range(4)]
                for ec in range(4):
                    nc.gpsimd.dma_start(out=wo_t[ec], in_=wo_d[ec, :, :, :])

                att_sb = {}
                for qb in range(NQB):
                    for g in range(KV):
                        att_sb[(g, qb)] = attsbp.tile(
                            [128, 512], BF, tag=f"at{g}_{qb}", name=f"at{g}_{qb}")

                for qb in range(NQB):
                    pr = {}
                    att_un = {}
                    for g in range(KV):
                        rhs_q = qT[g][:, :, qb * 128:(qb + 1) * 128]
                        ps_att = ps_attp.tile([128, 512], F32, tag="att",
                                              name="ps_att")
                        for j in range(NJ):
                            kb = qb + j
                            ksl = slice(kb * 128, (kb + 1) * 128)
                            ps_sc = ps_scp.tile([128, 512], F32, tag="sc",
                                                name="ps_sc")
                            nc.tensor.matmul(ps_sc, kT[g][:, ksl], rhs_q,
                                             start=True, stop=True)
                            p = probsp.tile([128, 512], BF, tag="pr", name="pr")
                            nc.scalar.activation(p, ps_sc, EXP, bias=zero_b[:, :])
                            if j == 0:
                                nc.vector.tensor_mul(p, p, m0)
                            elif j == NJ - 1:
                                nc.vector.tensor_mul(p, p, m4)
                            pr[(g, j)] = p
                            nc.tensor.matmul(
                                ps_att, v_sb[kb][:, g * 128:(g + 1) * 128],
                                p, start=(j == 0), stop=(j == NJ - 1))
                        au = attup.tile([128, 512], F32, tag="attu", name="att_un")
                        nc.scalar.activation(au, ps_att, COPY)
                        att_un[g] = au

                    # denominators: col-tiled M=1 matmuls, 4 groups concurrent
                    ps_den = ps_denp.tile([128, 512], F32, tag="den",
                                          name="ps_den")
                    for j in range(NJ):
                        for g in range(KV):
                            nc.tensor.matmul(
                                ps_den[32 * g:32 * g + 1, :], ones_b,
                                pr[(g, j)], start=(j == 0), stop=(j == NJ - 1),
                                tile_position=(0, 32 * g),
                                skip_group_check=True)
                    # per-group [1,512] denominator at partition 0 (PSUM
                    # reads may be partition-shifted; partition_broadcast on
                    # hw only honors base partition 0)
                    for g in range(KV):
                        den_g = small.tile([1, 512], F32, tag="den_g",
                                           name="den_g")
                        nc.vector.scalar_tensor_tensor(
                            den_g, ps_den[32 * g:32 * g + 1, :], 1.0,
                            corr[0:1, qb * 512:(qb + 1) * 512], MULT, SUB)
                        rec_g = small.tile([1, 512], F32, tag="rec_g",
                                           name="rec_g")
                        nc.vector.reciprocal_approx_fast(out=rec_g, in_=den_g)
                        bc = bcp.tile([128, 512], F32, tag="bc", name="bc")
                        nc.gpsimd.partition_broadcast(bc, rec_g)
                        nc.gpsimd.tensor_mul(att_sb[(g, qb)], att_un[g], bc)

                # output projection (bf16); per-block DMA out
                for ec in range(4):
                    for qb in range(NQB):
                        ps = ps_proj.tile([128, 512], F32, tag="proj", name="pso")
                        for h in range(H):
                            g, hg = h // NREP, h % NREP
                            nc.tensor.matmul(
                                ps, att_sb[(g, qb)][:, hg * 128:(hg + 1) * 128],
                                wo_t[ec][:, h, :], start=(h == 0),
                                stop=(h == H - 1))
                        ob = obufp.tile([128, 512], BF, tag="ob", name="ob")
                        nc.vector.tensor_copy(ob, ps)
                        nc.sync.dma_start(
                            out=out_d[qb * 128:(qb + 1) * 128,
                                      ec * 512:(ec + 1) * 512],
                            in_=ob)

    nc.compile()
    return nc


def _prep_inputs(x, Wq, Wk, Wv, Wo):
    """Host-side prep: shard + transpose + quantize. Returns list of in_maps."""
    x = np.asarray(x, np.float32)
    Wq = np.asarray(Wq, np.float32)
    Wk = np.asarray(Wk, np.float32)
    Wv = np.asarray(Wv, np.float32)
    Wo = np.asarray(Wo, np.float32)

    # weights: shared across cores
    wq8 = np.ascontiguousarray(
        (Wq * WS).reshape(H, 128, EC, 128).transpose(0, 3, 2, 1)).astype(FP8)
    wk8 = np.ascontiguousarray(
        (Wk * WS).reshape(KV, 128, EC, 128).transpose(3, 2, 0, 1)).astype(FP8)
    wv = np.ascontiguousarray(
        Wv.reshape(KV * 128, EC, 128).transpose(2, 1, 0)).astype(BF16)
    wo = np.ascontiguousarray(
        Wo.reshape(4, 512, H, 128).transpose(0, 3, 2, 1)).astype(BF16)

    inv_freq = 1.0 / (THETA ** (np.arange(0, D, 2, dtype=np.float32) / D))  # [64]
    scale = np.float32(1.0 / np.sqrt(D))

    kp = np.arange(128)[:, None]
    qf = np.arange(128)[None, :]
    m0 = np.tile((kp > qf).astype(np.float32), (1, NREP)).astype(BF16)
    m4 = np.tile((kp <= qf).astype(np.float32), (1, NREP)).astype(BF16)

    in_maps = []
    for c in range(NCORES):
        b, ch = c // 4, c % 4
        q0 = ch * Q
        lo = q0 - WINDOW
        xc = np.zeros((TH, E), np.float32)
        xc[max(0, -lo):] = x[b, max(0, lo):q0 + Q]
        xT = np.ascontiguousarray(xc.T.reshape(EC, 128, TH).transpose(1, 0, 2))
        xt8 = xT.astype(FP8)  # [128, EC, TH]
        xtb = np.ascontiguousarray(
            xT.reshape(128, EC, 4, 256).transpose(2, 0, 1, 3)).astype(BF16)

        pos_k = np.arange(lo, q0 + Q, dtype=np.float32)
        ang_k = inv_freq[:, None] * pos_k[None, :]
        pos_q = np.arange(q0, q0 + Q, dtype=np.float32)
        ang_q = inv_freq[:, None] * pos_q[None, :]
        ckh, skh = np.cos(ang_k), np.sin(ang_k)
        cqh, sqh = np.cos(ang_q), np.sin(ang_q)
        ck = (np.concatenate([ckh, ckh], 0) / WS).astype(np.float32)
        sk = (np.concatenate([skh, -skh], 0) / WS).astype(np.float32)
        cq = (np.concatenate([cqh, cqh], 0) * (scale / WS)).astype(np.float32)
        sq = (np.concatenate([sqh, -sqh], 0) * (scale / WS)).astype(np.float32)

        # denominator correction: padded keys inside the window contribute
        # exp(0) = 1 each (only for sequence-start chunks). -1 on unused rows
        # keeps the fast reciprocal away from 1/0 on garbage lanes.
        if ch == 0:
            q_l = WINDOW + np.arange(Q)
            cnt = np.maximum(0, (TH - 1) - q_l).astype(np.float32)
        else:
            cnt = np.zeros(Q, np.float32)
        row = np.tile(cnt.reshape(NQB, 1, 128), (1, NREP, 1)).reshape(NQB * 512)
        corr = np.full((128, NQB * 512), -1.0, np.float32)
        for g in range(KV):
            corr[32 * g] = row

        in_maps.append({
            "xt8": xt8, "xtb": xtb,
            "wq8": wq8, "wk8": wk8, "wv": wv, "wo": wo,
            "ck": ck, "sk": sk, "cq": cq, "sq": sq,
            "mask0": m0, "mask4": m4,
            "corr": corr,
        })
    return in_maps


def _get_nc():
    if "nc" not in _CACHE:
        _CACHE["nc"] = _build_bass()
    return _CACHE["nc"]


def run(inputs, trace=False, **kw):
    nc = _get_nc()
    in_maps = _prep_inputs(**inputs)
    res = run_bass_kernel_spmd(nc, in_maps, core_ids=list(range(NCORES)),
                               trace=trace, **kw)
    out = np.empty((B, T, E), np.float32)
    for c in range(NCORES):
        b, ch = c // 4, c % 4
        out[b, ch * Q:(ch + 1) * Q] = res.results[c]["out"].astype(np.float32)
    return out, res


def kernel(**inputs):
    out, _ = run(inputs, trace=False)
    return out
